# revision 2
# baseline (speedup 1.0000x reference)
"""Trainium2 Bass kernel: 3x3 stride-1 pad-1 Conv2D forward + bias.

Full problem: x (32,32,128,128) f32, kernels (64,288) f32, bias (64,1) f32
-> out (32,64,128,128) f32.

Sharding: data-parallel over batch — 4 images per core on 8 NeuronCores,
weights/bias replicated. No collectives needed (forward only).

Per-core algorithm (pair-row formulation, all 128 PE columns used):
  PE output columns = (2 adjacent output rows x 64 channels); contraction =
  (4-row input window x 32 channels) = all 128 partitions. For an even output
  row rho, the 4-row window x[*, rho-1 : rho+3] feeds both row rho (taps
  j=0,1,2 of the window) and row rho+1 (taps j=1,2,3). Per kernel column dc
  one PSUM-accumulated matmul covers 4 row-pairs (rhs free = (4 pairs, 128 w)
  = 512); 3 dc passes accumulate. PE streams 3*16*512 = 24576 cols/image —
  half of the 64-wide formulation.

  SBUF layout R[32*j + c, h, :] = x[c, h+j-1] horizontally padded (width
  130), bf16: 4 vertically-shifted replicas in 4 contiguous 32-partition
  blocks. Slots j=1,2 load directly from HBM (two DMA queues); slots j=0,3
  are built by on-chip shifted copies (DVE / GpSimd) from the loaded slots,
  keeping HBM input traffic at 2 image-copies instead of 4.

  Bias folds into the PSUM->SBUF eviction: DVE tensor_scalar_add / ACT
  activation-add with a per-partition [128,1] bias operand (bias[k]
  replicated to both column groups). Output is staged and DMA'd as bf16
  (halves HBM write traffic; host upcasts to fp32 — ~0.4% rel err vs the
  2e-2 budget). Stores ride the SWDGE (gpsimd) queue so they never
  head-of-line block the HWDGE rings carrying the latency-critical loads.
"""

import numpy as np
import ml_dtypes

import concourse.bass as bass
import concourse.mybir as mybir
import concourse.tile as tile
from concourse import bacc
from concourse.bass_utils import run_bass_kernel_spmd

N_CORES = 8
B, C, H, W = 32, 32, 128, 128
K = 64
B_LOC = B // N_CORES  # images per core
WP = W + 2  # padded row pitch

DT = "bf16"
REP_MODE = "hbm2"  # how slots j=0..3 of R are produced: hbm1/hbm2/hbm3/hbm4

_DT_MAP = {
    "bf16": (mybir.dt.bfloat16, ml_dtypes.bfloat16),
    "fp32r": (mybir.dt.float32r, np.float32),
}


def _build(
    dt_name: str = DT,
    reps: int = 1,
    bench_io: bool = False,
    rep_mode: str = REP_MODE,
    n_bufs: int = 3,
):
    mdt, _ = _DT_MAP[dt_name]
    f32 = mybir.dt.float32
    bf16 = mybir.dt.bfloat16

    nc = bacc.Bacc("TRN2", target_bir_lowering=False, debug=False)
    xp = nc.dram_tensor("xp", [B_LOC, C, H, WP], mdt, kind="ExternalInput")
    wt = nc.dram_tensor("wt", [128, 3, 128], mdt, kind="ExternalInput")
    bs = nc.dram_tensor("bs", [128, 1], f32, kind="ExternalInput")
    if bench_io:
        # timing variant: big output stays in device DRAM; tiny external out
        out = nc.dram_tensor("obuf", [B_LOC, K, H, W], bf16)
        tout = nc.dram_tensor("tout", [1, 1], bf16, kind="ExternalOutput")
    else:
        out = nc.dram_tensor("out", [B_LOC, K, H, W], bf16, kind="ExternalOutput")

    with tile.TileContext(nc) as tc:
        with (
            tc.tile_pool(name="const", bufs=1) as const_pool,
            tc.tile_pool(name="xrep", bufs=1) as xrep_pool,
            tc.tile_pool(name="psum", bufs=1, space="PSUM") as psum_pool,
            tc.tile_pool(name="ostage", bufs=n_bufs) as ostage_pool,
        ):
            wsb = const_pool.tile([128, 3, 128], mdt, name="wsb")
            nc.sync.dma_start(wsb[:], wt[:])
            bsb = const_pool.tile([128, 1], f32, name="bsb")
            nc.sync.dma_start(bsb[:], bs[:])

            # Persistent replicated-image buffers. Slot block j holds the
            # image shifted vertically by j-1; rows outside the valid range
            # stay zero (memset once; per-image writes never touch them).
            Rs = []
            for i in range(n_bufs):
                R = xrep_pool.tile([128, H, WP], mdt, name=f"R{i}", tag=f"R{i}")
                nc.vector.memset(R[0:32, 0:1, :], 0.0)  # j=0: x[c,-1]
                nc.vector.memset(R[64:96, H - 1 : H, :], 0.0)  # j=2: x[c,H]
                nc.vector.memset(R[96:128, H - 2 : H, :], 0.0)  # j=3: x[c,H..H+1]
                Rs.append(R)

            for rep in range(reps):
                for n in range(B_LOC):
                    R = Rs[n % n_bufs]
                    # slot j=1 (unshifted) always loads from HBM
                    nc.sync.dma_start(R[32:64, :, :], xp[n])
                    if rep_mode in ("hbm2", "hbm3", "hbm4"):
                        # slot j=2 (shift +1) from HBM
                        nc.scalar.dma_start(R[64:96, 0 : H - 1, :], xp[n, :, 1:H, :])
                    else:
                        # j=2 from slot 1: R[64+c,h] = x[c,h+1] = R[32+c,h+1]
                        nc.gpsimd.tensor_copy(
                            R[64:96, 0 : H - 1, :], R[32:64, 1:H, :]
                        )
                    if rep_mode in ("hbm3", "hbm4"):
                        # slot j=0 (shift -1) from HBM
                        [nc.sync, nc.scalar][n % 2].dma_start(
                            R[0:32, 1:H, :], xp[n, :, 0 : H - 1, :]
                        )
                    else:
                        # j=0 from slot 1: R[c,h] = x[c,h-1] = R[32+c,h-1]
                        nc.vector.tensor_copy(R[0:32, 1:H, :], R[32:64, 0 : H - 1, :])
                    if rep_mode == "hbm4":
                        # slot j=3 (shift +2) from HBM
                        [nc.scalar, nc.sync][n % 2].dma_start(
                            R[96:128, 0 : H - 2, :], xp[n, :, 2:H, :]
                        )
                    elif rep_mode == "hbm1":
                        # j=3 from slot 1: R[96+c,h] = x[c,h+2] = R[32+c,h+2]
                        nc.scalar.copy(R[96:128, 0 : H - 2, :], R[32:64, 2:H, :])
                    else:
                        # j=3 from slot 2: R[96+c,h] = x[c,h+2] = R[64+c,h+1]
                        nc.gpsimd.tensor_copy(
                            R[96:128, 0 : H - 2, :], R[64:96, 1 : H - 1, :]
                        )

                    # DRAM view matching ost: h = 64*hf + 8*b + 2*hp + par
                    o_n = out[n].rearrange(
                        "k (hf b hp par) w -> hf par k b hp w", hf=2, b=8, par=2
                    )
                    for half in range(2):
                        psums = [
                            psum_pool.tile(
                                [128, 512], f32, name=f"ps{b}", tag=f"ps{b}"
                            )
                            for b in range(8)
                        ]
                        for dc in range(3):
                            for b in range(8):
                                r0 = 64 * half + 8 * b
                                nc.tensor.matmul(
                                    psums[b][:, :],
                                    lhsT=wsb[:, dc, :],
                                    rhs=R[:, r0 : r0 + 8 : 2, dc : dc + W],
                                    start=(dc == 0),
                                    stop=(dc == 2),
                                )
                        ost = ostage_pool.tile(
                            [128, 8, 512], bf16, name="ost", tag="ost"
                        )
                        for b in range(8):
                            # psum->SBUF eviction with fused bias add,
                            # alternating DVE/ACT
                            if b % 2 == 0:
                                nc.vector.tensor_scalar_add(
                                    ost[:, b, :], psums[b][:, :], bsb[:, :]
                                )
                            else:
                                nc.scalar.add(ost[:, b, :], psums[b][:, :], bsb[:, :])
                        # stores ride SWDGE so they never head-of-line block
                        # the HWDGE rings carrying the input loads
                        for par in range(2):
                            nc.gpsimd.dma_start(
                                o_n[half, par],
                                ost[64 * par : 64 * par + 64, :, :],
                            )

            if bench_io:
                # read tout from obuf so the per-rep output DMAs stay live
                nc.sync.dma_start(tout[:], out[0, 0, 0, 0:1])

    nc.compile()
    return nc


def _prep_weights(kernels: np.ndarray, dt_name: str = DT):
    _, npdt = _DT_MAP[dt_name]
    w4 = kernels.reshape(K, C, 3, 3).astype(np.float32)  # [k, c, dr, dc]
    wt = np.zeros((4, C, 3, 128), np.float32)  # [j, c, dc, col]
    for j in range(4):
        for par in range(2):
            dr = j - par
            if 0 <= dr <= 2:
                # wt[j, c, dc, 64*par + k] = w4[k, c, dr, dc]
                wt[j, :, :, 64 * par : 64 * par + 64] = np.transpose(
                    w4[:, :, dr, :], (1, 2, 0)
                )
    return np.ascontiguousarray(wt.reshape(128, 3, 128).astype(npdt))


def _prep_bias(bias: np.ndarray):
    bs = np.empty((128, 1), np.float32)
    bs[0:64, 0] = bias.reshape(K).astype(np.float32)
    bs[64:128, 0] = bias.reshape(K).astype(np.float32)
    return bs


def _prep_x(x: np.ndarray, dt_name: str = DT):
    _, npdt = _DT_MAP[dt_name]
    xp = np.zeros((B, C, H, WP), npdt)
    xp[:, :, :, 1 : W + 1] = x.astype(npdt)
    return xp


_NC_CACHE: dict[tuple, object] = {}


def _run(x, kernels, bias, dt_name: str = DT, reps: int = 1, trace: bool = False):
    key = (dt_name, reps)
    if key not in _NC_CACHE:
        _NC_CACHE[key] = _build(dt_name, reps)
    nc = _NC_CACHE[key]

    xp = _prep_x(np.asarray(x), dt_name)
    wt = _prep_weights(np.asarray(kernels), dt_name)
    bs = _prep_bias(np.asarray(bias))
    in_maps = [
        {"xp": xp[c * B_LOC : (c + 1) * B_LOC], "wt": wt, "bs": bs}
        for c in range(N_CORES)
    ]
    kw = {"trace": True} if trace else {}
    res = run_bass_kernel_spmd(nc, in_maps, list(range(N_CORES)), **kw)
    full = np.concatenate(
        [res.results[c]["out"].astype(np.float32) for c in range(N_CORES)], axis=0
    )
    return full, res


def kernel(x, kernels, bias):
    full, _ = _run(x, kernels, bias)
    return full


# revision 6
# speedup vs baseline: 2.5749x; 2.5749x over previous
"""Trainium2 Bass kernel: 3x3 stride-1 pad-1 Conv2D forward + bias.

Full problem: x (32,32,128,128) f32, kernels (64,288) f32, bias (64,1) f32
-> out (32,64,128,128) f32.

Sharding: data-parallel over batch — 4 images per core on 8 NeuronCores,
weights/bias replicated. No collectives needed (forward only).

Per-core algorithm (pair-row formulation, all 128 PE columns used):
  PE output columns = (2 adjacent output rows x 64 channels); contraction =
  (4-row input window x 32 channels) = all 128 partitions. For an even output
  row rho, the 4-row window x[*, rho-1 : rho+3] feeds both row rho (taps
  j=0,1,2 of the window) and row rho+1 (taps j=1,2,3). Per kernel column dc
  one PSUM-accumulated matmul covers 4 row-pairs (rhs free = (4 pairs, 128 w)
  = 512); 3 dc passes accumulate. PE streams 3*16*512 = 24576 cols/image —
  half of the 64-wide formulation.

  SBUF layout R[32*j + c, h, :] = x[c, h+j-1] horizontally padded (width
  130), bf16: 4 vertically-shifted replicas in 4 contiguous 32-partition
  blocks. Slots j=1,2 load from HBM on the two HWDGE rings; slots j=0,3 are
  shifted copies on DVE (the only engine with a 16-bit 2x path; Pool/ACT
  copies cost ~3-5x more), keeping HBM input traffic at 2 image-copies.

  PSUM is split into two grouped tiles (2 banks + 6 banks) so each
  eviction is ONE wide instruction (amortizes the per-instruction PSUM
  access overhead): DVE evicts the small group, ACT the big one, both with
  a fused per-partition bias add. Output rows interleave parity (even rows
  in PE columns 0-63, odd in 64-127), so the DRAM output tensor is
  parity-major [B_LOC, 128, 64, W] — each partition's 8KB half-image block
  is contiguous (1 DMA descriptor per partition, one store per half) — and
  the host de-interleaves. Output is staged and DMA'd as bf16 (halves HBM
  write traffic; host upcasts to fp32). Stores ride the SWDGE (gpsimd)
  queue so they never head-of-line block the HWDGE rings.
"""

import numpy as np
import ml_dtypes

import concourse.bass as bass
import concourse.mybir as mybir
import concourse.tile as tile
from concourse import bacc
from concourse.bass_utils import run_bass_kernel_spmd

N_CORES = 8
B, C, H, W = 32, 32, 128, 128
K = 64
B_LOC = B // N_CORES  # images per core
WP = W + 2  # padded row pitch

DT = "bf16"
NB_A = 2  # PSUM banks evicted by DVE (rest go to ACT)

_DT_MAP = {
    "bf16": (mybir.dt.bfloat16, ml_dtypes.bfloat16),
    "fp32r": (mybir.dt.float32r, np.float32),
}


def _build(
    dt_name: str = DT,
    reps: int = 1,
    bench_io: bool = False,
    rep_mode: str = "hbm2",
    n_bufs: int = 3,
    nb_a: int = NB_A,
    copy_op: str = "copy",  # 'copy' | 'ts_add' (tensor_scalar_add + 0.0)
    evict: str = "split",  # 'split' (DVE A + ACT B) | 'act' (both on ACT)
):
    mdt, _ = _DT_MAP[dt_name]
    f32 = mybir.dt.float32
    bf16 = mybir.dt.bfloat16
    nb_b = 8 - nb_a

    nc = bacc.Bacc("TRN2", target_bir_lowering=False, debug=False)
    xp = nc.dram_tensor("xp", [B_LOC, C, H, WP], mdt, kind="ExternalInput")
    wt = nc.dram_tensor("wt", [128, 3, 128], mdt, kind="ExternalInput")
    bs = nc.dram_tensor("bs", [128, 1], f32, kind="ExternalInput")
    # parity-major output: [n, 64*par + k, h//2, w], h = 2*(h//2) + par
    if bench_io:
        out = nc.dram_tensor("obuf", [B_LOC, 128, H // 2, W], bf16)
        tout = nc.dram_tensor("tout", [1, 1], bf16, kind="ExternalOutput")
    else:
        out = nc.dram_tensor(
            "out", [B_LOC, 128, H // 2, W], bf16, kind="ExternalOutput"
        )

    with tile.TileContext(nc) as tc:
        with (
            tc.tile_pool(name="const", bufs=1) as const_pool,
            tc.tile_pool(name="xrep", bufs=1) as xrep_pool,
            tc.tile_pool(name="psum", bufs=1, space="PSUM") as psum_pool,
            tc.tile_pool(name="ostage", bufs=n_bufs) as ostage_pool,
        ):
            wsb = const_pool.tile([128, 3, 128], mdt, name="wsb")
            nc.sync.dma_start(wsb[:], wt[:])
            bsb = const_pool.tile([128, 1], f32, name="bsb")
            nc.sync.dma_start(bsb[:], bs[:])

            # Persistent replicated-image buffers. Slot block j holds the
            # image shifted vertically by j-1; rows outside the valid range
            # stay zero (memset once; per-image writes never touch them).
            Rs = []
            for i in range(n_bufs):
                R = xrep_pool.tile([128, H, WP], mdt, name=f"R{i}", tag=f"R{i}")
                nc.vector.memset(R[0:32, 0:1, :], 0.0)  # j=0: x[c,-1]
                nc.vector.memset(R[64:96, H - 1 : H, :], 0.0)  # j=2: x[c,H]
                nc.vector.memset(R[96:128, H - 2 : H, :], 0.0)  # j=3: x[c,H..]
                Rs.append(R)

            def vcopy(dst, src):
                if copy_op == "ts_add":
                    nc.vector.tensor_scalar_add(dst, src, 0.0)
                else:
                    nc.vector.tensor_copy(dst, src)

            for rep in range(reps):
                for n in range(B_LOC):
                    R = Rs[n % n_bufs]
                    # slot j=1 always loads from HBM (SP HWDGE ring)
                    nc.sync.dma_start(R[32:64, :, :], xp[n])
                    if rep_mode == "hbm1":
                        # all other slots copied from slot 1 on DVE
                        vcopy(R[0:32, 1:H, :], R[32:64, 0 : H - 1, :])
                        vcopy(R[64:96, 0 : H - 1, :], R[32:64, 1:H, :])
                        vcopy(R[96:128, 0 : H - 2, :], R[32:64, 2:H, :])
                    elif rep_mode == "hbm4":
                        nc.scalar.dma_start(R[64:96, 0 : H - 1, :], xp[n, :, 1:H, :])
                        nc.sync.dma_start(R[0:32, 1:H, :], xp[n, :, 0 : H - 1, :])
                        nc.scalar.dma_start(R[96:128, 0 : H - 2, :], xp[n, :, 2:H, :])
                    else:  # hbm2
                        nc.scalar.dma_start(R[64:96, 0 : H - 1, :], xp[n, :, 1:H, :])
                        # j=0 from slot 1: R[c,h] = x[c,h-1] = R[32+c,h-1]
                        vcopy(R[0:32, 1:H, :], R[32:64, 0 : H - 1, :])
                        # j=3 from slot 2: R[96+c,h] = x[c,h+2] = R[64+c,h+1]
                        vcopy(R[96:128, 0 : H - 2, :], R[64:96, 1 : H - 1, :])

                    # DRAM view: [hf, (par k), hh, w], h = 64hf + 8b + 2hp + par
                    o2 = out[n].rearrange("p (hf hh) w -> hf p hh w", hf=2)
                    for half in range(2):
                        psA = psum_pool.tile([128, nb_a, 512], f32, name="psA", tag="psA")
                        psB = psum_pool.tile([128, nb_b, 512], f32, name="psB", tag="psB")
                        for dc in range(3):
                            for b in range(8):
                                r0 = 64 * half + 8 * b
                                tgt = (
                                    psA[:, b, :] if b < nb_a else psB[:, b - nb_a, :]
                                )
                                nc.tensor.matmul(
                                    tgt,
                                    lhsT=wsb[:, dc, :],
                                    rhs=R[:, r0 : r0 + 8 : 2, dc : dc + W],
                                    start=(dc == 0),
                                    stop=(dc == 2),
                                )
                        ost = ostage_pool.tile(
                            [128, 8, 512], bf16, name="ost", tag="ost"
                        )
                        # PSUM->SBUF eviction, one wide instruction per
                        # group, fused per-partition bias add
                        if evict == "act":
                            nc.scalar.add(ost[:, 0:nb_a, :], psA[:, :, :], bsb[:, :])
                        else:
                            nc.vector.tensor_scalar_add(
                                ost[:, 0:nb_a, :], psA[:, :, :], bsb[:, :]
                            )
                        nc.scalar.add(ost[:, nb_a:8, :], psB[:, :, :], bsb[:, :])
                        # one store per half: each partition writes its 8KB
                        # contiguous half-image block (1 descriptor each)
                        nc.gpsimd.dma_start(o2[half], ost[:, :, :])

            if bench_io:
                # read tout from obuf so the per-rep output DMAs stay live
                nc.sync.dma_start(tout[:], out[0, 0, 0, 0:1])

    nc.compile()
    return nc


def _prep_weights(kernels: np.ndarray, dt_name: str = DT):
    _, npdt = _DT_MAP[dt_name]
    w4 = kernels.reshape(K, C, 3, 3).astype(np.float32)  # [k, c, dr, dc]
    wt = np.zeros((4, C, 3, 128), np.float32)  # [j, c, dc, col]
    for j in range(4):
        for par in range(2):
            dr = j - par
            if 0 <= dr <= 2:
                # wt[j, c, dc, 64*par + k] = w4[k, c, dr, dc]
                wt[j, :, :, 64 * par : 64 * par + 64] = np.transpose(
                    w4[:, :, dr, :], (1, 2, 0)
                )
    return np.ascontiguousarray(wt.reshape(128, 3, 128).astype(npdt))


def _prep_bias(bias: np.ndarray):
    bs = np.empty((128, 1), np.float32)
    bs[0:64, 0] = bias.reshape(K).astype(np.float32)
    bs[64:128, 0] = bias.reshape(K).astype(np.float32)
    return bs


def _prep_x(x: np.ndarray, dt_name: str = DT):
    _, npdt = _DT_MAP[dt_name]
    xp = np.zeros((B, C, H, WP), npdt)
    xp[:, :, :, 1 : W + 1] = x.astype(npdt)
    return xp


def _unpack_out(res: np.ndarray) -> np.ndarray:
    # [n, 64*par + k, h//2, w] -> [n, k, h, w]
    r = res.astype(np.float32).reshape(B_LOC, 2, K, H // 2, W)
    return np.ascontiguousarray(r.transpose(0, 2, 3, 1, 4)).reshape(B_LOC, K, H, W)


_NC_CACHE: dict[tuple, object] = {}


def _run(x, kernels, bias, dt_name: str = DT, reps: int = 1, trace: bool = False):
    key = (dt_name, reps)
    if key not in _NC_CACHE:
        _NC_CACHE[key] = _build(dt_name, reps)
    nc = _NC_CACHE[key]

    xp = _prep_x(np.asarray(x), dt_name)
    wt = _prep_weights(np.asarray(kernels), dt_name)
    bs = _prep_bias(np.asarray(bias))
    in_maps = [
        {"xp": xp[c * B_LOC : (c + 1) * B_LOC], "wt": wt, "bs": bs}
        for c in range(N_CORES)
    ]
    kw = {"trace": True} if trace else {}
    res = run_bass_kernel_spmd(nc, in_maps, list(range(N_CORES)), **kw)
    full = np.concatenate(
        [_unpack_out(res.results[c]["out"]) for c in range(N_CORES)], axis=0
    )
    return full, res


def kernel(x, kernels, bias):
    full, _ = _run(x, kernels, bias)
    return full


# revision 9
# speedup vs baseline: 3.3240x; 1.2909x over previous
"""Trainium2 Bass kernel: 3x3 stride-1 pad-1 Conv2D forward + bias.

Full problem: x (32,32,128,128) f32, kernels (64,288) f32, bias (64,1) f32
-> out (32,64,128,128) f32.

Sharding: data-parallel over batch — 4 images per core on 8 NeuronCores,
weights/bias replicated. No collectives needed (forward only).

Per-core algorithm (pair-row formulation, all 128 PE columns used):
  PE output columns = (2 adjacent output rows x 64 channels); contraction =
  (4-row input window x 32 channels) = all 128 partitions. For an even output
  row rho, the 4-row window x[*, rho-1 : rho+3] feeds both row rho (taps
  j=0,1,2 of the window) and row rho+1 (taps j=1,2,3). Per kernel column dc
  one PSUM-accumulated matmul covers 4 row-pairs (rhs free = (4 pairs, 128 w)
  = 512); 3 dc passes accumulate. PE streams 3*16*512 = 24576 cols/image —
  half of the 64-wide formulation.

  SBUF layout R[32*j + c, h, :] = x[c, h+j-1] horizontally padded (width
  130), bf16: 4 vertically-shifted replicas in 4 contiguous 32-partition
  blocks. Slots j=1,2 load from HBM on the two HWDGE rings; slots j=0,3 are
  shifted copies on DVE (the only engine with a 16-bit 2x path; Pool/ACT
  copies cost ~3-5x more), keeping HBM input traffic at 2 image-copies.

  PSUM is split into two grouped tiles (2 banks + 6 banks) so each
  eviction is ONE wide instruction (amortizes the per-instruction PSUM
  access overhead): DVE evicts the small group, ACT the big one, both with
  a fused per-partition bias add. Output rows interleave parity (even rows
  in PE columns 0-63, odd in 64-127), so the DRAM output tensor is
  parity-major [B_LOC, 128, 64, W] — each partition's 8KB half-image block
  is contiguous (1 DMA descriptor per partition, one store per half) — and
  the host de-interleaves. Output is staged and DMA'd as bf16 (halves HBM
  write traffic; host upcasts to fp32). Stores ride the SWDGE (gpsimd)
  queue so they never head-of-line block the HWDGE rings.
"""

import numpy as np
import ml_dtypes

import concourse.bass as bass
import concourse.mybir as mybir
import concourse.tile as tile
from concourse import bacc
from concourse.bass_utils import run_bass_kernel_spmd

N_CORES = 8
B, C, H, W = 32, 32, 128, 128
K = 64
B_LOC = B // N_CORES  # images per core
WP = W + 2  # padded row pitch

DT = "bf16"
NB_A = 2  # PSUM banks evicted by DVE (rest go to ACT)

_DT_MAP = {
    "bf16": (mybir.dt.bfloat16, ml_dtypes.bfloat16),
    "fp32r": (mybir.dt.float32r, np.float32),
}


def _build(
    dt_name: str = DT,
    reps: int = 1,
    bench_io: bool = False,
    rep_mode: str = "hbm2",
    n_bufs: int = 3,
    nb_a: int = NB_A,
    copy_op: str = "copy",  # 'copy' | 'ts_add' (tensor_scalar_add + 0.0)
    evict: str = "act",  # 'split' (DVE A + ACT B) | 'act' (both on ACT)
    order: str = "dc_major",  # 'dc_major' | 'bank_major'
):
    mdt, _ = _DT_MAP[dt_name]
    f32 = mybir.dt.float32
    bf16 = mybir.dt.bfloat16
    nb_b = 8 - nb_a

    nc = bacc.Bacc("TRN2", target_bir_lowering=False, debug=False)
    xp = nc.dram_tensor("xp", [B_LOC, C, H, WP], mdt, kind="ExternalInput")
    wt = nc.dram_tensor("wt", [128, 3, 128], mdt, kind="ExternalInput")
    bs = nc.dram_tensor("bs", [128, 1], f32, kind="ExternalInput")
    # parity-major output: [n, 64*par + k, h//2, w], h = 2*(h//2) + par
    if bench_io:
        out = nc.dram_tensor("obuf", [B_LOC, 128, H // 2, W], bf16)
        tout = nc.dram_tensor("tout", [1, 1], bf16, kind="ExternalOutput")
    else:
        out = nc.dram_tensor(
            "out", [B_LOC, 128, H // 2, W], bf16, kind="ExternalOutput"
        )

    with tile.TileContext(nc) as tc:
        with (
            tc.tile_pool(name="const", bufs=1) as const_pool,
            tc.tile_pool(name="xrep", bufs=1) as xrep_pool,
            tc.tile_pool(name="psum", bufs=1, space="PSUM") as psum_pool,
            tc.tile_pool(name="ostage", bufs=n_bufs) as ostage_pool,
        ):
            wsb = const_pool.tile([128, 3, 128], mdt, name="wsb")
            nc.sync.dma_start(wsb[:], wt[:])
            bsb = const_pool.tile([128, 1], f32, name="bsb")
            nc.sync.dma_start(bsb[:], bs[:])

            # Persistent replicated-image buffers. Slot block j holds the
            # image shifted vertically by j-1; rows outside the valid range
            # stay zero (memset once; per-image writes never touch them).
            Rs = []
            for i in range(n_bufs):
                R = xrep_pool.tile([128, H, WP], mdt, name=f"R{i}", tag=f"R{i}")
                nc.vector.memset(R[0:32, 0:1, :], 0.0)  # j=0: x[c,-1]
                nc.vector.memset(R[64:96, H - 1 : H, :], 0.0)  # j=2: x[c,H]
                nc.vector.memset(R[96:128, H - 2 : H, :], 0.0)  # j=3: x[c,H..]
                Rs.append(R)

            def vcopy(dst, src):
                if copy_op == "ts_add":
                    nc.vector.tensor_scalar_add(dst, src, 0.0)
                else:
                    nc.vector.tensor_copy(dst, src)

            for rep in range(reps):
                for n in range(B_LOC):
                    R = Rs[n % n_bufs]
                    # slot j=1 always loads from HBM (SP HWDGE ring)
                    nc.sync.dma_start(R[32:64, :, :], xp[n])
                    if rep_mode == "hbm1":
                        # all other slots copied from slot 1 on DVE
                        vcopy(R[0:32, 1:H, :], R[32:64, 0 : H - 1, :])
                        vcopy(R[64:96, 0 : H - 1, :], R[32:64, 1:H, :])
                        vcopy(R[96:128, 0 : H - 2, :], R[32:64, 2:H, :])
                    elif rep_mode == "hbm4":
                        nc.scalar.dma_start(R[64:96, 0 : H - 1, :], xp[n, :, 1:H, :])
                        nc.sync.dma_start(R[0:32, 1:H, :], xp[n, :, 0 : H - 1, :])
                        nc.scalar.dma_start(R[96:128, 0 : H - 2, :], xp[n, :, 2:H, :])
                    else:  # hbm2
                        nc.scalar.dma_start(R[64:96, 0 : H - 1, :], xp[n, :, 1:H, :])
                        # j=0 from slot 1: R[c,h] = x[c,h-1] = R[32+c,h-1]
                        vcopy(R[0:32, 1:H, :], R[32:64, 0 : H - 1, :])
                        # j=3 from slot 2: R[96+c,h] = x[c,h+2] = R[64+c,h+1]
                        vcopy(R[96:128, 0 : H - 2, :], R[64:96, 1 : H - 1, :])

                    # DRAM view: [hf, (par k), hh, w], h = 64hf + 8b + 2hp + par
                    o2 = out[n].rearrange("p (hf hh) w -> hf p hh w", hf=2)
                    for half in range(2):
                        psA = psum_pool.tile([128, nb_a, 512], f32, name="psA", tag="psA")
                        psB = psum_pool.tile([128, nb_b, 512], f32, name="psB", tag="psB")
                        if order == "bank_major":
                            mm_iter = [(dc, b) for b in range(8) for dc in range(3)]
                        else:
                            mm_iter = [(dc, b) for dc in range(3) for b in range(8)]
                        for dc, b in mm_iter:
                            r0 = 64 * half + 8 * b
                            tgt = psA[:, b, :] if b < nb_a else psB[:, b - nb_a, :]
                            nc.tensor.matmul(
                                tgt,
                                lhsT=wsb[:, dc, :],
                                rhs=R[:, r0 : r0 + 8 : 2, dc : dc + W],
                                start=(dc == 0),
                                stop=(dc == 2),
                            )
                        ost = ostage_pool.tile(
                            [128, 8, 512], bf16, name="ost", tag="ost"
                        )
                        # PSUM->SBUF eviction, one wide instruction per
                        # group, fused per-partition bias add
                        if evict == "act":
                            nc.scalar.add(ost[:, 0:nb_a, :], psA[:, :, :], bsb[:, :])
                        else:
                            nc.vector.tensor_scalar_add(
                                ost[:, 0:nb_a, :], psA[:, :, :], bsb[:, :]
                            )
                        nc.scalar.add(ost[:, nb_a:8, :], psB[:, :, :], bsb[:, :])
                        # one store per half: each partition writes its 8KB
                        # contiguous half-image block (1 descriptor each)
                        nc.gpsimd.dma_start(o2[half], ost[:, :, :])

            if bench_io:
                # read tout from obuf so the per-rep output DMAs stay live
                nc.sync.dma_start(tout[:], out[0, 0, 0, 0:1])

    nc.compile()
    return nc


def _prep_weights(kernels: np.ndarray, dt_name: str = DT):
    _, npdt = _DT_MAP[dt_name]
    w4 = kernels.reshape(K, C, 3, 3).astype(np.float32)  # [k, c, dr, dc]
    wt = np.zeros((4, C, 3, 128), np.float32)  # [j, c, dc, col]
    for j in range(4):
        for par in range(2):
            dr = j - par
            if 0 <= dr <= 2:
                # wt[j, c, dc, 64*par + k] = w4[k, c, dr, dc]
                wt[j, :, :, 64 * par : 64 * par + 64] = np.transpose(
                    w4[:, :, dr, :], (1, 2, 0)
                )
    return np.ascontiguousarray(wt.reshape(128, 3, 128).astype(npdt))


def _prep_bias(bias: np.ndarray):
    bs = np.empty((128, 1), np.float32)
    bs[0:64, 0] = bias.reshape(K).astype(np.float32)
    bs[64:128, 0] = bias.reshape(K).astype(np.float32)
    return bs


def _prep_x(x: np.ndarray, dt_name: str = DT):
    _, npdt = _DT_MAP[dt_name]
    xp = np.zeros((B, C, H, WP), npdt)
    xp[:, :, :, 1 : W + 1] = x.astype(npdt)
    return xp


def _unpack_out(res: np.ndarray) -> np.ndarray:
    # [n, 64*par + k, h//2, w] -> [n, k, h, w]
    r = res.astype(np.float32).reshape(B_LOC, 2, K, H // 2, W)
    return np.ascontiguousarray(r.transpose(0, 2, 3, 1, 4)).reshape(B_LOC, K, H, W)


_NC_CACHE: dict[tuple, object] = {}


def _run(x, kernels, bias, dt_name: str = DT, reps: int = 1, trace: bool = False):
    key = (dt_name, reps)
    if key not in _NC_CACHE:
        _NC_CACHE[key] = _build(dt_name, reps)
    nc = _NC_CACHE[key]

    xp = _prep_x(np.asarray(x), dt_name)
    wt = _prep_weights(np.asarray(kernels), dt_name)
    bs = _prep_bias(np.asarray(bias))
    in_maps = [
        {"xp": xp[c * B_LOC : (c + 1) * B_LOC], "wt": wt, "bs": bs}
        for c in range(N_CORES)
    ]
    kw = {"trace": True} if trace else {}
    res = run_bass_kernel_spmd(nc, in_maps, list(range(N_CORES)), **kw)
    full = np.concatenate(
        [_unpack_out(res.results[c]["out"]) for c in range(N_CORES)], axis=0
    )
    return full, res


def kernel(x, kernels, bias):
    full, _ = _run(x, kernels, bias)
    return full
